# revision 1
# baseline (speedup 1.0000x reference)
"""Trainium2 Bass kernel for the neural-backflow problem — Fourier method.

backflow_c[i] = sum_j H_c^{spin}(r_i - r_j), H_c(D) = wrap(D)_c * F(|wrap(D)|)
is a C-infinity function on the period-10 torus, so it factorizes through a
truncated 3D Fourier series (|k|_inf <= P = 12):

  sum_j H_c(r_i - r_j) = sum_q  M_c[q] * phi_q(r_i),
  M_c[q] = Wtil_c[q] * S[perm_c(q)],     S[q] = sum_j phi_q(r_j),

with phi_q the separable real trig product basis (per axis: cos(k*theta),
sin(k*theta), theta = omega*(v-5) in [-pi, pi)) and Wtil_c host-precomputed
amplitude tensors (FFT of H_c at kernel-call time; rs-independent).

Device pipeline per core (512 electrons' rows):
  1. features: one Sin activation (theta in range) + Chebyshev recurrence
  2. GXY[j, qx, qy] = phix*phiy per 128-j block (broadcast-AP mults)
  3. S[qz, qx*qy] via PE matmuls (f32r, full rate), separate up/dn spins
  4. M_c = WtilA*S_A + WtilB*S_B (crossed half-reads implement the C/S swap)
  5. back-transform: T1 = zfT.T @ M_c on PE; B = sum(T1 * GXY_i) fused
     multiply-reduce on DVE, seeded with rs_i so out = rs + backflow directly.

Self-pairs contribute exactly zero (every retained mode is odd in D_c).
"""

import numpy as np

import concourse.bass as bass
import concourse.mybir as mybir
from concourse.tile import TileContext
from concourse.bass_utils import run_bass_kernel_spmd

L = 10.0
N = 4096
N_UP = 2048
NCORES = 8
ROWS = N // NCORES          # 512 rows per core
NBLK = N // 128             # 32 j-blocks
NIB = ROWS // 128           # 4 i-blocks per core
P = 12                      # max mode per axis
Q = P + 1                   # 13: cos slots k=0..P (also sin slots k=0..P)
R = 2 * Q                   # 26 basis funcs per axis (S_0 slot identically 0)
RR = R * R                  # 676
HALF = Q * R                # 338 (qx half x full qy)
OMEGA = 2.0 * np.pi / L

F32 = mybir.dt.float32
F32R = mybir.dt.float32r
BF16 = mybir.dt.bfloat16
AOP = mybir.AluOpType
AF = mybir.ActivationFunctionType

LAST_RESULTS = None
_CACHED = {}

# ---------------------------------------------------------------- host side


def _neural_decayed_np(x, w1, b1, wo, bo):
    x_cut = L / 2
    xn = np.clip(x / x_cut, 0.0, 1.0 - 1e-05)
    decay = np.exp(1.0 - 1.0 / (1.0 - xn ** 2))
    z = decay[:, None] @ w1 + b1
    h = z / (1.0 + np.exp(-z))
    return ((h @ wo + bo).ravel()) * decay


def _amp_tensors(w1, b1, wo, bo, G=96):
    """A_c[kx,ky,kz] (k=0..P): amplitude of sin(k_c w D_c)*prod cos(k w D)
    in H_c(D) = wrap(D)_c * F(|wrap(D)|)."""
    w1 = np.asarray(w1, np.float64).reshape(1, -1)
    b1 = np.asarray(b1, np.float64).ravel()
    wo = np.asarray(wo, np.float64).reshape(-1, 1)
    bo = np.asarray(bo, np.float64).ravel()
    g = np.arange(G) * (L / G)
    X, Y, Z = np.meshgrid(g, g, g, indexing="ij")
    wrap = lambda u: (u + L / 2) % L - L / 2
    mx, my, mz = wrap(X), wrap(Y), wrap(Z)
    r = np.sqrt(mx ** 2 + my ** 2 + mz ** 2)
    F = _neural_decayed_np(r.ravel(), w1, b1, wo, bo).reshape(r.shape)
    ks = np.arange(Q)
    ang = np.outer(g, ks) * OMEGA
    Bc = np.cos(ang) * (2.0 / G)
    Bc[:, 0] *= 0.5
    Bs = np.sin(ang) * (2.0 / G)
    A = []
    for c, m in enumerate((mx, my, mz)):
        H = m * F
        mats = [Bc, Bc, Bc]
        mats[c] = Bs
        t = np.einsum("abc,ax->xbc", H, mats[0], optimize=True)
        t = np.einsum("xbc,by->xyc", t, mats[1], optimize=True)
        t = np.einsum("xyc,cz->xyz", t, mats[2], optimize=True)
        A.append(t)
    return A


def _w_dev(A):
    """Device amplitude tensors W_dev[c][qz, qx, qy] (f32), including the
    sin-slot sign on the swap axis and the half-swap permutation baked in
    for c=0 (qx) and c=1 (qy).  Basis slots: q<Q: cos(k), q>=Q: sin(k=q-Q)."""
    kq = np.concatenate([np.arange(Q), np.arange(Q)])      # k per slot
    sin_slot = np.arange(R) >= Q
    out = []
    for c in range(3):
        Wc = A[c][np.ix_(kq, kq, kq)]                      # [qx,qy,qz]
        sgn = np.where(sin_slot, -1.0, 1.0)
        if c == 0:
            Wc = Wc * sgn[:, None, None]
        elif c == 1:
            Wc = Wc * sgn[None, :, None]
        else:
            Wc = Wc * sgn[None, None, :]
        # zero the identically-zero sin(k=0) slots for cleanliness
        z = np.zeros(R, bool)
        z[Q] = True
        Wc[z, :, :] = 0.0
        Wc[:, z, :] = 0.0
        Wc[:, :, z] = 0.0
        Wc = np.transpose(Wc, (2, 0, 1))                   # [qz, qx, qy]
        perm = np.concatenate([np.arange(Q, R), np.arange(Q)])
        if c == 0:
            Wc = Wc[:, perm, :]
        elif c == 1:
            Wc = Wc[:, :, perm]
        out.append(Wc.astype(np.float32))
    return out


def host_prepare(rs, same_w1, same_b1, same_wo, same_bo,
                 diff_w1, diff_b1, diff_wo, diff_bo):
    """Build per-core input maps (without repstag)."""
    rs = np.ascontiguousarray(np.asarray(rs, np.float32))
    A_same = _amp_tensors(same_w1, same_b1, same_wo, same_bo)
    A_diff = _amp_tensors(diff_w1, diff_b1, diff_wo, diff_bo)
    W_same = _w_dev(A_same)    # list of 3 [R, R, R] = [qz, qx, qy]
    W_diff = _w_dev(A_diff)
    wS = np.ascontiguousarray(np.stack(W_same, axis=1).reshape(R, 3, R, R))
    wD = np.ascontiguousarray(np.stack(W_diff, axis=1).reshape(R, 3, R, R))
    rsP = np.ascontiguousarray(
        rs.reshape(NBLK, 128, 3).transpose(1, 0, 2))       # [128, 32, 3]
    idn = np.ascontiguousarray(np.eye(128, dtype=np.float32))
    in_maps = []
    for core in range(NCORES):
        up = (core * ROWS) < N_UP
        own = list(range(core * NIB, (core + 1) * NIB))
        same = [b for b in (range(0, 16) if up else range(16, 32))
                if b not in own]
        other = list(range(16, 32) if up else range(0, 16))
        order = own + same + other
        # device blocks 0..3 = this core's rows; 0..15 = same-spin electrons
        in_maps.append({
            "rsP": np.ascontiguousarray(rsP[:, order, :]),
            "wA": wS,                 # S over device blocks 0..15: same spin
            "wB": wD,                 # S over device blocks 16..31: other
            "idn": idn,
        })
    return in_maps


# -------------------------------------------------------------- device side


def _build_program(reps=1):
    nc = bass.Bass()
    rsP = nc.declare_dram_parameter("rsP", [128, NBLK, 3], F32, isOutput=False)
    wA = nc.declare_dram_parameter("wA", [R, 3, R, R], F32, isOutput=False)
    wB = nc.declare_dram_parameter("wB", [R, 3, R, R], F32, isOutput=False)
    idn = nc.declare_dram_parameter("idn", [128, 128], F32, isOutput=False)
    repstag = nc.declare_dram_parameter("repstag", [reps, 1], F32, isOutput=False)
    outp = nc.declare_dram_parameter("out", [ROWS, 3], F32, isOutput=True)

    # SPMD: the host reorders rsP's blocks per core so device blocks 0..3 are
    # this core's own rows and 0..15 are same-spin electrons (see host_prepare)

    with TileContext(nc) as tc:
        with (
            tc.tile_pool(name="const", bufs=1) as cpool,
            tc.tile_pool(name="feat", bufs=1) as fpool,
            tc.tile_pool(name="work", bufs=2) as wpool,
            tc.tile_pool(name="small", bufs=2) as spool,
            tc.tile_pool(name="psum", bufs=1, space="PSUM") as ppool,
            tc.tile_pool(name="psum2", bufs=2, space="PSUM") as ppool2,
        ):
            RSP = cpool.tile([128, NBLK, 3], F32, tag="RSP")
            nc.gpsimd.dma_start(out=RSP[:], in_=rsP[:, :, :])
            WA = cpool.tile([R, 3, R, R], F32, tag="WA")
            nc.gpsimd.dma_start(out=WA[:], in_=wA[:, :, :, :])
            WB = cpool.tile([R, 3, R, R], F32, tag="WB")
            nc.gpsimd.dma_start(out=WB[:], in_=wB[:, :, :, :])
            IDN = cpool.tile([128, 128], F32, tag="IDN")
            nc.gpsimd.dma_start(out=IDN[:], in_=idn[:, :])
            IDNb = cpool.tile([128, 128], BF16, tag="IDNb")
            nc.gpsimd.tensor_copy(IDNb[:], IDN[:])
            rtag = cpool.tile([1, 1], F32, tag="rtag")
            nc.gpsimd.dma_start(out=rtag[:], in_=repstag[reps - 1:reps, :])

            for rep in range(reps):
                # ---------------- stage 0: features --------------------
                # FE[p, b, c, q]: q<Q: cos(k theta), q>=Q: sin(k theta)
                FE = fpool.tile([128, NBLK, 3, R], F32, tag="FE")
                TH = spool.tile([128, NBLK, 3, 1], F32, tag="TH")
                nc.gpsimd.tensor_scalar(
                    TH[:], RSP[:].unsqueeze(3), float(OMEGA),
                    float(-5.0 * OMEGA), AOP.mult, AOP.add)
                TH2 = spool.tile([128, NBLK, 3, 1], F32, tag="TH2")
                nc.gpsimd.tensor_scalar(TH2[:], TH[:], 0.5, None, AOP.mult)
                # C_0 = 1, S_0 = 0
                nc.gpsimd.memset(FE[:, :, :, 0:1], 1.0)
                nc.gpsimd.memset(FE[:, :, :, Q:Q + 1], 0.0)
                # S_1 = sin(theta); SH = sin(theta/2); C_1 = 1 - 2 SH^2
                nc.scalar.activation(FE[:, :, :, Q + 1:Q + 2], TH[:], AF.Sin)
                SH = spool.tile([128, NBLK, 3, 1], F32, tag="SH")
                nc.scalar.activation(SH[:], TH2[:], AF.Sin)
                SQ = spool.tile([128, NBLK, 3, 1], F32, tag="SQ")
                nc.vector.tensor_tensor(SQ[:], SH[:], SH[:], AOP.mult)
                nc.vector.tensor_scalar(
                    FE[:, :, :, 1:2], SQ[:], -2.0, 1.0, AOP.mult, AOP.add)
                TW = spool.tile([128, NBLK, 3, 1], F32, tag="TW")
                nc.vector.tensor_scalar(
                    TW[:], FE[:, :, :, 1:2], 2.0, None, AOP.mult)
                # Chebyshev recurrence, split blocks across DVE / GpSimd
                HB = NBLK // 2
                halves = [(nc.vector, slice(0, HB)), (nc.gpsimd, slice(HB, NBLK))]
                for k in range(2, Q):
                    for eng, bs in halves:
                        # V_k = 2c1 * V_{k-1} - V_{k-2}  for both C and S rows
                        t = wpool.tile([128, HB, 3, 2], F32,
                                       name=f"rec{bs.start}", tag=f"rec{bs.start}")
                        v1 = FE[:, bs, :, k - 1::Q]   # cols {k-1, Q+k-1}
                        v2 = FE[:, bs, :, k - 2::Q]
                        eng.tensor_tensor(
                            t[:], TW[:, bs, :, :].broadcast_to([128, HB, 3, 2]),
                            v1, AOP.mult)
                        eng.tensor_tensor(FE[:, bs, :, k::Q], t[:], v2,
                                          AOP.subtract)

                # bf16 copy of z-features (matmul operands must be bf16)
                FEZb = fpool.tile([128, NBLK, R], BF16, tag="FEZb")
                nc.vector.tensor_copy(FEZb[:], FE[:, :, 2, :])

                # ---------------- stage 0b: zfT transposes ------------
                zfT = []
                zfTx = []
                for ib in range(NIB):
                    tp = ppool2.tile([R, 128], BF16, name=f"tp{ib}", tag="tp")
                    nc.tensor.matmul(tp[:], FEZb[:, ib, :], IDNb[:],
                                     is_transpose=True)
                    zn = fpool.tile([R, 128], BF16, name=f"zfT{ib}", tag=f"zfT{ib}")
                    nc.scalar.copy(zn[:], tp[:])
                    zfT.append(zn)
    # z-features with C/S halves swapped, then transposed
                    fzx = wpool.tile([128, R], BF16, name=f"fzx{ib}", tag="fzx")
                    nc.gpsimd.tensor_copy(fzx[:, 0:Q], FEZb[:, ib, Q:R])
                    nc.gpsimd.tensor_copy(fzx[:, Q:R], FEZb[:, ib, 0:Q])
                    tpx = ppool2.tile([R, 128], BF16, name=f"tpx{ib}", tag="tp")
                    nc.tensor.matmul(tpx[:], fzx[:], IDNb[:], is_transpose=True)
                    zx = fpool.tile([R, 128], BF16, name=f"zfTx{ib}", tag=f"zfTx{ib}")
                    nc.scalar.copy(zx[:], tpx[:])
                    zfTx.append(zx)

                # ---------------- stage 1+2: GXY + S ------------------
                SPS = [[ppool.tile([R, Q, R], F32, name=f"S{s}{h}", tag=f"S{s}{h}")
                        for h in range(2)] for s in range(2)]
                GI = []
                DVE_BLOCKS = 18
                for b in range(NBLK):
                    s = 0 if b < NBLK // 2 else 1
                    if b < NIB:
                        g = fpool.tile([128, R, R], BF16, name=f"GI{b}",
                                       tag=f"GI{b}")
                        GI.append(g)
                    else:
                        g = wpool.tile([128, R, R], BF16, name="gxy", tag="gxy")
                    eng = nc.vector if (b % NBLK) < DVE_BLOCKS else nc.gpsimd
                    eng.tensor_tensor(
                        g[:],
                        FE[:, b, 0, :].unsqueeze(2).broadcast_to([128, R, R]),
                        FE[:, b, 1, :].unsqueeze(1).broadcast_to([128, R, R]),
                        AOP.mult)
                    first = b % (NBLK // 2) == 0
                    last = b % (NBLK // 2) == NBLK // 2 - 1
                    for h in range(2):
                        nc.tensor.matmul(
                            SPS[s][h][:],
                            FEZb[:, b, :],
                            g[:, h * Q:(h + 1) * Q, :],
                            start=first, stop=last)

                # S psum -> sbuf (ACT copies)
                SS = []
                for s in range(2):
                    t = spool.tile([R, 2, Q, R], F32, name=f"SS{s}", tag=f"SS{s}")
                    for h in range(2):
                        nc.scalar.copy(t[:, h], SPS[s][h][:])
                    SS.append(t)

                # ---------------- stage 3: M_c ------------------------
                # SS layout [qz, (2,Q)=qx, qy]; W layout [qz, c, qx, qy]
                M = []
                for c in range(3):
                    m = spool.tile([R, 2, Q, R], BF16, name=f"M{c}", tag=f"M{c}")
                    t1 = wpool.tile([R, 2, Q, R], F32, name="mt1", tag="mt1")
                    t2 = wpool.tile([R, 2, Q, R], F32, name="mt2", tag="mt2")
                    WAc = WA[:, c].rearrange("z (hx q) y -> z hx q y", hx=2)
                    WBc = WB[:, c].rearrange("z (hx q) y -> z hx q y", hx=2)
                    eng = nc.vector if c != 1 else nc.gpsimd
                    if c == 0:
                        # crossed half read on qx
                        for h in range(2):
                            eng.tensor_tensor(t1[:, h], WAc[:, h], SS[0][:, 1 - h],
                                              AOP.mult)
                            eng.tensor_tensor(t2[:, h], WBc[:, h], SS[1][:, 1 - h],
                                              AOP.mult)
                            eng.tensor_tensor(m[:, h], t1[:, h], t2[:, h], AOP.add)
                    elif c == 1:
                        # crossed read on qy inner (2, Q) split
                        WAc5 = WA[:, c].rearrange(
                            "z x (hy q) -> z x hy q", hy=2)
                        WBc5 = WB[:, c].rearrange(
                            "z x (hy q) -> z x hy q", hy=2)
                        SS05 = [SS[0].rearrange("z hx q (hy p) -> z (hx q) hy p", hy=2),
                                SS[1].rearrange("z hx q (hy p) -> z (hx q) hy p", hy=2)]
                        m5 = m.rearrange("z hx q (hy p) -> z (hx q) hy p", hy=2)
                        t15 = t1.rearrange("z hx q (hy p) -> z (hx q) hy p", hy=2)
                        t25 = t2.rearrange("z hx q (hy p) -> z (hx q) hy p", hy=2)
                        for s in range(2):
                            eng.tensor_tensor(t15[:, :, s], WAc5[:, :, s],
                                              SS05[0][:, :, 1 - s], AOP.mult)
                            eng.tensor_tensor(t25[:, :, s], WBc5[:, :, s],
                                              SS05[1][:, :, 1 - s], AOP.mult)
                        eng.tensor_tensor(m5[:], t15[:], t25[:], AOP.add)
                    else:
                        eng.tensor_tensor(t1[:], WAc, SS[0][:], AOP.mult)
                        eng.tensor_tensor(t2[:], WBc, SS[1][:], AOP.mult)
                        eng.tensor_tensor(m[:], t1[:], t2[:], AOP.add)
                    M.append(m)

                # ---------------- stage 4: back-transform -------------
                for ib in range(NIB):
                    res = spool.tile([128, 3], F32, name=f"res{ib}", tag="res")
                    gi2 = GI[ib][:].rearrange("p a b -> p (a b)")
                    for c in range(3):
                        lhsT = (zfT[ib] if c != 2 else zfTx[ib])
                        accs = []
                        for h in range(2):
                            t1p = ppool2.tile([128, HALF], F32,
                                              name=f"T1_{ib}{c}{h}", tag="T1")
                            nc.tensor.matmul(
                                t1p[:], lhsT[:], M[c][:, h],
                                start=True, stop=True)
                            scr = wpool.tile([128, HALF], F32, name="scr",
                                             tag="scr")
                            acc = spool.tile([128, 1], F32,
                                             name=f"acc{ib}{c}{h}",
                                             tag=f"acc{h}")
                            nc.vector.scalar_tensor_tensor(
                                scr[:], t1p[:], 0.0,
                                gi2[:, h * HALF:(h + 1) * HALF],
                                AOP.bypass, AOP.mult, accum_out=acc[:])
                            accs.append(acc)
                        # res_c = (acc0 + rs_i_c) + acc1
                        nc.vector.scalar_tensor_tensor(
                            res[:, c:c + 1], accs[0][:], RSP[:, ib, c:c + 1],
                            accs[1][:], AOP.add, AOP.add)
                    nc.sync.dma_start(
                        out=outp[ib * 128:(ib + 1) * 128, :], in_=res[:])
    return nc


def _split_multi_waits(bir_json: bytes) -> bytes:
    """Walrus rejects >1 sync wait per instruction; hoist extras onto
    same-engine NoOps immediately before (same blocking semantics)."""
    import json as _json
    d = _json.loads(bir_json)
    for fn in d["functions"]:
        for blk in fn["blocks"]:
            new_insts = []
            for inst in blk["instructions"]:
                si = inst.get("sync_info")
                waits = (si or {}).get("on_wait") or []
                if len(waits) > 1:
                    for i, w in enumerate(waits[:-1]):
                        new_insts.append({
                            "debug": inst.get("debug", 0),
                            "engine": inst["engine"],
                            "ins": [], "outs": [],
                            "name": f"{inst['name']}-w{i}",
                            "opcode": "NoOp",
                            "text_hint": "split_wait",
                            "sync_info": {"on_update": [], "on_wait": [w]},
                        })
                    si["on_wait"] = [waits[-1]]
                new_insts.append(inst)
            blk["instructions"] = new_insts
    return _json.dumps(d).encode()


def _get_program(reps=1):
    if reps not in _CACHED:
        nc = _build_program(reps)
        orig = nc.to_json_bytes
        nc.to_json_bytes = lambda: _split_multi_waits(orig())
        _CACHED[reps] = nc
    return _CACHED[reps]


def kernel(rs, same_w1, same_b1, same_wo, same_bo,
           diff_w1, diff_b1, diff_wo, diff_bo):
    global LAST_RESULTS
    in_maps = host_prepare(rs, same_w1, same_b1, same_wo, same_bo,
                           diff_w1, diff_b1, diff_wo, diff_bo)
    for im in in_maps:
        im["repstag"] = np.zeros((1, 1), np.float32)
    nc = _get_program()
    LAST_RESULTS = run_bass_kernel_spmd(nc, in_maps, list(range(NCORES)))
    outs = [np.asarray(LAST_RESULTS.results[i]["out"]) for i in range(NCORES)]
    return np.concatenate(outs, axis=0).astype(np.float32)



# revision 3
# speedup vs baseline: 22.4406x; 22.4406x over previous
"""Trainium2 Bass kernel for the neural-backflow problem — Fourier method, P=7.

backflow_c[i] = sum_j H_c^{spin}(r_i - r_j), H_c(D) = wrap(D)_c * F(|wrap(D)|)
is a C-infinity function on the period-10 torus, so it factorizes through a
truncated 3D Fourier series (|k|_inf <= P = 7):

  sum_j H_c(r_i - r_j) = sum_q  M_c[q] * phi_q(r_i),
  M_c[q] = Wtil_c[q] * S[perm_c(q)],     S[q] = sum_j phi_q(r_j),

with phi_q the separable real trig product basis (per axis: cos(k*theta),
sin(k*theta), theta = omega*(v-5) in [-pi, pi)) and Wtil_c host-precomputed
amplitude tensors (FFT of H_c at kernel-call time; rs-independent).

Device pipeline per core (512 electrons' rows):
  1. features: one Sin activation (theta in range) + Chebyshev recurrence
  2. GXY[j, qx, qy] = phix*phiy per 128-j block (broadcast-AP mults)
  3. S[qz, qx*qy] via PE matmuls (256-wide, fits one PSUM bank at R=16),
     separate up/dn spins
  4. M_c = WtilA*S_A + WtilB*S_B (crossed half-reads implement the C/S swap)
  5. back-transform: T1 = zfT.T @ M_c on PE; B = sum(T1 * GXY_i) fused
     multiply-reduce on DVE, seeded with rs_i so out = rs + backflow directly.

Self-pairs contribute exactly zero (every retained mode is odd in D_c).
"""

import numpy as np

import concourse.bass as bass
import concourse.mybir as mybir
from concourse.tile import TileContext
from concourse.bass_utils import run_bass_kernel_spmd

L = 10.0
N = 4096
N_UP = 2048
NCORES = 8
ROWS = N // NCORES          # 512 rows per core
NBLK = N // 128             # 32 j-blocks
NIB = ROWS // 128           # 4 i-blocks per core
P = 6                       # max mode per axis
Q = P + 1                   # 8: cos slots k=0..P (also sin slots k=0..P)
R = 2 * Q                   # 16 basis funcs per axis (S_0 slot identically 0)
RR = R * R                  # 256
HALF = Q * R                # 128 (qx half x full qy)
OMEGA = 2.0 * np.pi / L

F32 = mybir.dt.float32
BF16 = mybir.dt.bfloat16
AOP = mybir.AluOpType
AF = mybir.ActivationFunctionType

LAST_RESULTS = None
_CACHED = {}

# ---------------------------------------------------------------- host side


def _neural_decayed_np(x, w1, b1, wo, bo):
    x_cut = L / 2
    xn = np.clip(x / x_cut, 0.0, 1.0 - 1e-05)
    decay = np.exp(1.0 - 1.0 / (1.0 - xn ** 2))
    z = decay[:, None] @ w1 + b1
    h = z / (1.0 + np.exp(-z))
    return ((h @ wo + bo).ravel()) * decay


def _amp_tensors(w1, b1, wo, bo, G=96):
    """A_c[kx,ky,kz] (k=0..P): amplitude of sin(k_c w D_c)*prod cos(k w D)
    in H_c(D) = wrap(D)_c * F(|wrap(D)|)."""
    w1 = np.asarray(w1, np.float64).reshape(1, -1)
    b1 = np.asarray(b1, np.float64).ravel()
    wo = np.asarray(wo, np.float64).reshape(-1, 1)
    bo = np.asarray(bo, np.float64).ravel()
    g = np.arange(G) * (L / G)
    X, Y, Z = np.meshgrid(g, g, g, indexing="ij")
    wrap = lambda u: (u + L / 2) % L - L / 2
    mx, my, mz = wrap(X), wrap(Y), wrap(Z)
    r = np.sqrt(mx ** 2 + my ** 2 + mz ** 2)
    F = _neural_decayed_np(r.ravel(), w1, b1, wo, bo).reshape(r.shape)
    ks = np.arange(Q)
    ang = np.outer(g, ks) * OMEGA
    Bc = np.cos(ang) * (2.0 / G)
    Bc[:, 0] *= 0.5
    Bs = np.sin(ang) * (2.0 / G)
    A = []
    for c, m in enumerate((mx, my, mz)):
        H = m * F
        mats = [Bc, Bc, Bc]
        mats[c] = Bs
        t = np.einsum("abc,ax->xbc", H, mats[0], optimize=True)
        t = np.einsum("xbc,by->xyc", t, mats[1], optimize=True)
        t = np.einsum("xyc,cz->xyz", t, mats[2], optimize=True)
        A.append(t)
    return A


def _w_dev(A):
    """Device amplitude tensors W_dev[c][qz, qx, qy] (f32), including the
    sin-slot sign on the swap axis and the half-swap permutation baked in
    for c=0 (qx) and c=1 (qy).  Basis slots: q<Q: cos(k), q>=Q: sin(k=q-Q)."""
    kq = np.concatenate([np.arange(Q), np.arange(Q)])      # k per slot
    sin_slot = np.arange(R) >= Q
    out = []
    for c in range(3):
        Wc = A[c][np.ix_(kq, kq, kq)]                      # [qx,qy,qz]
        sgn = np.where(sin_slot, -1.0, 1.0)
        if c == 0:
            Wc = Wc * sgn[:, None, None]
        elif c == 1:
            Wc = Wc * sgn[None, :, None]
        else:
            Wc = Wc * sgn[None, None, :]
        # zero the identically-zero sin(k=0) slots for cleanliness
        z = np.zeros(R, bool)
        z[Q] = True
        Wc[z, :, :] = 0.0
        Wc[:, z, :] = 0.0
        Wc[:, :, z] = 0.0
        Wc = np.transpose(Wc, (2, 0, 1))                   # [qz, qx, qy]
        perm = np.concatenate([np.arange(Q, R), np.arange(Q)])
        if c == 0:
            Wc = Wc[:, perm, :]
        elif c == 1:
            Wc = Wc[:, :, perm]
        out.append(Wc.astype(np.float32))
    return out


def host_prepare(rs, same_w1, same_b1, same_wo, same_bo,
                 diff_w1, diff_b1, diff_wo, diff_bo):
    """Build per-core input maps (without repstag)."""
    rs = np.ascontiguousarray(np.asarray(rs, np.float32))
    A_same = _amp_tensors(same_w1, same_b1, same_wo, same_bo)
    A_diff = _amp_tensors(diff_w1, diff_b1, diff_wo, diff_bo)
    W_same = _w_dev(A_same)    # list of 3 [R, R, R] = [qz, qx, qy]
    W_diff = _w_dev(A_diff)
    wS = np.ascontiguousarray(np.stack(W_same, axis=1).reshape(R, 3, R, R))
    wD = np.ascontiguousarray(np.stack(W_diff, axis=1).reshape(R, 3, R, R))
    rsP = np.ascontiguousarray(
        rs.reshape(NBLK, 128, 3).transpose(1, 0, 2))       # [128, 32, 3]
    idn = np.ascontiguousarray(np.eye(128, dtype=np.float32))
    in_maps = []
    for core in range(NCORES):
        up = (core * ROWS) < N_UP
        own = list(range(core * NIB, (core + 1) * NIB))
        same = [b for b in (range(0, 16) if up else range(16, 32))
                if b not in own]
        other = list(range(16, 32) if up else range(0, 16))
        order = own + same + other
        # device blocks 0..3 = this core's rows; 0..15 = same-spin electrons
        in_maps.append({
            "rsP": np.ascontiguousarray(rsP[:, order, :]),
            "wA": wS,                 # S over device blocks 0..15: same spin
            "wB": wD,                 # S over device blocks 16..31: other
            "idn": idn,
        })
    return in_maps


# -------------------------------------------------------------- device side


def _build_program(reps=1):
    nc = bass.Bass()
    rsP = nc.declare_dram_parameter("rsP", [128, NBLK, 3], F32, isOutput=False)
    wA = nc.declare_dram_parameter("wA", [R, 3, R, R], F32, isOutput=False)
    wB = nc.declare_dram_parameter("wB", [R, 3, R, R], F32, isOutput=False)
    idn = nc.declare_dram_parameter("idn", [128, 128], F32, isOutput=False)
    repstag = nc.declare_dram_parameter("repstag", [reps, 1], F32, isOutput=False)
    outp = nc.declare_dram_parameter("out", [128, NIB, 3], F32, isOutput=True)

    # SPMD: the host reorders rsP's blocks per core so device blocks 0..3 are
    # this core's own rows and 0..15 are same-spin electrons (see host_prepare)

    with TileContext(nc) as tc:
        with (
            tc.tile_pool(name="const", bufs=1) as cpool,
            tc.tile_pool(name="feat", bufs=1) as fpool,
            tc.tile_pool(name="work", bufs=2) as wpool,
            tc.tile_pool(name="small", bufs=2) as spool,
            tc.tile_pool(name="gxy", bufs=3) as gpool,
            tc.tile_pool(name="psum", bufs=1, space="PSUM") as ppool,
            tc.tile_pool(name="psum2", bufs=2, space="PSUM") as ppool2,
        ):
            # const loads on the HWDGE engines (SP + ACT): ~0.6us each vs
            # ~1us software descriptor generation per DMA on gpsimd.
            RSP = cpool.tile([128, NBLK, 3], F32, tag="RSP")
            nc.sync.dma_start(out=RSP[:], in_=rsP[:, :, :])
            IDN = cpool.tile([128, 128], F32, tag="IDN")
            nc.scalar.dma_start(out=IDN[:], in_=idn[:, :])
            # W tensors are needed late (M stage) — issue on the otherwise
            # idle-early Pool queue (SWDGE) to keep HWDGE free for RSP
            WA = cpool.tile([R, 3, R, R], F32, tag="WA")
            nc.gpsimd.dma_start(out=WA[:], in_=wA[:, :, :, :])
            WB = cpool.tile([R, 3, R, R], F32, tag="WB")
            nc.gpsimd.dma_start(out=WB[:], in_=wB[:, :, :, :])
            IDNb = cpool.tile([128, 128], BF16, tag="IDNb")
            nc.gpsimd.tensor_copy(IDNb[:], IDN[:])
            rtag = cpool.tile([1, 1], F32, tag="rtag")
            nc.sync.dma_start(out=rtag[:], in_=repstag[reps - 1:reps, :])
            # padded z-feature staging for one-shot transposes: block ib sits
            # at partition-aligned slot 32*ib after the transpose
            FEZP = cpool.tile([128, NIB, 32], BF16, tag="FEZP")
            nc.gpsimd.memset(FEZP[:], 0.0)
            FZXP = cpool.tile([128, NIB, 32], BF16, tag="FZXP")
            nc.gpsimd.memset(FZXP[:], 0.0)
            # per-partition scale/bias constants for the base Sin activations
            SOM = cpool.tile([128, 1], F32, tag="SOM")
            nc.gpsimd.memset(SOM[:], float(OMEGA))
            BN5O = cpool.tile([128, 1], F32, tag="BN5O")
            nc.gpsimd.memset(BN5O[:], float(-5.0 * OMEGA))
            SOM2 = cpool.tile([128, 1], F32, tag="SOM2")
            nc.gpsimd.memset(SOM2[:], float(0.5 * OMEGA))
            BN5O2 = cpool.tile([128, 1], F32, tag="BN5O2")
            nc.gpsimd.memset(BN5O2[:], float(-2.5 * OMEGA))

            for rep in range(reps):
                # ---------------- stage 0: features --------------------
                # FE[p, b, c, q]: q<Q: cos(k theta), q>=Q: sin(k theta)
                FE = fpool.tile([128, NBLK, 3, R], F32, tag="FE")
                # C_0 = 1, S_0 = 0
                nc.gpsimd.memset(FE[:, :, :, 0:1], 1.0)
                nc.gpsimd.memset(FE[:, :, :, Q:Q + 1], 0.0)
                # S_1 = sin(w(v-5)), SH = sin(w(v-5)/2) directly off RSP via
                # the activation's per-partition scale/bias; C_1 = 1 - 2 SH^2
                nc.scalar.activation(FE[:, :, :, Q + 1:Q + 2],
                                     RSP[:].unsqueeze(3), AF.Sin,
                                     bias=BN5O[:], scale=SOM[:])
                SH = spool.tile([128, NBLK, 3, 1], F32, tag="SH")
                nc.scalar.activation(SH[:], RSP[:].unsqueeze(3), AF.Sin,
                                     bias=BN5O2[:], scale=SOM2[:])
                SQ = spool.tile([128, NBLK, 3, 1], F32, tag="SQ")
                nc.vector.scalar_tensor_tensor(
                    SQ[:], SH[:], -2.0, SH[:], AOP.mult, AOP.mult)
                nc.vector.tensor_scalar(
                    FE[:, :, :, 1:2], SQ[:], 1.0, None, AOP.add)
                DW = 20       # DVE chain blocks; Pool gets the rest
                TW = spool.tile([128, NBLK, 3, 1], F32, tag="TW")
                nc.gpsimd.tensor_scalar(
                    TW[:, DW:NBLK], FE[:, DW:NBLK, :, 1:2], 2.0, None,
                    AOP.mult)
                # Chebyshev recurrence, split blocks across DVE / GpSimd.
                # DVE fuses the 2*C1 factor via scalar_tensor_tensor; Pool
                # lacks that form and multiplies by the TW = 2*C1 tensor.
                halves = [(nc.vector, slice(0, DW)), (nc.gpsimd, slice(DW, NBLK))]
                for k in range(2, Q):
                    for eng, bs in halves:
                        # V_k = 2c1 * V_{k-1} - V_{k-2}  for both C and S rows
                        w = bs.stop - bs.start
                        t = wpool.tile([128, w, 3, 2], F32,
                                       name=f"rec{bs.start}", tag=f"rec{bs.start}")
                        v1 = FE[:, bs, :, k - 1::Q]   # cols {k-1, Q+k-1}
                        v2 = FE[:, bs, :, k - 2::Q]
                        if eng is nc.vector:
                            eng.scalar_tensor_tensor(
                                t[:], v1, 2.0,
                                FE[:, bs, :, 1:2].broadcast_to([128, w, 3, 2]),
                                AOP.mult, AOP.mult)
                        else:
                            eng.tensor_tensor(
                                t[:],
                                TW[:, bs, :, :].broadcast_to([128, w, 3, 2]),
                                v1, AOP.mult)
                        eng.tensor_tensor(FE[:, bs, :, k::Q], t[:], v2,
                                          AOP.subtract)

                # bf16 copies, split per chain half so downstream work on
                # the DVE half's blocks starts before the Pool half finishes
                FEZb = fpool.tile([128, NBLK, R], BF16, tag="FEZb")
                FEXYb = fpool.tile([128, NBLK, 2, R], BF16, tag="FEXYb")
                nc.scalar.copy(FEZb[:, 0:DW, :], FE[:, 0:DW, 2, :])
                nc.vector.tensor_copy(FEXYb[:, 0:DW], FE[:, 0:DW, 0:2, :])
                nc.scalar.copy(FEZb[:, DW:NBLK, :], FE[:, DW:NBLK, 2, :])
                nc.scalar.copy(FEXYb[:, DW:NBLK], FE[:, DW:NBLK, 0:2, :])

                dve_gxy = list(range(0, 16)) + list(range(27, NBLK))
                pool_gxy = list(range(16, 27))
                GI = []
                gtiles = {}

                def emit_gxy(eng, blocks):
                    for b in blocks:
                        if b < NIB:
                            g = fpool.tile([128, R, R], BF16, name=f"GI{b}",
                                           tag=f"GI{b}")
                            GI.append(g)
                        else:
                            tg = "gxyv" if eng is nc.vector else "gxyp"
                            g = gpool.tile([128, R, R], BF16, name=tg, tag=tg)
                        gtiles[b] = g
                        eng.tensor_tensor(
                            g[:],
                            FEXYb[:, b, 0, :].unsqueeze(2)
                            .broadcast_to([128, R, R]),
                            FEXYb[:, b, 1, :].unsqueeze(1)
                            .broadcast_to([128, R, R]),
                            AOP.mult)

                emit_gxy(nc.vector, dve_gxy)
                emit_gxy(nc.gpsimd, pool_gxy)

                # ---------------- stage 0b: zfT transposes ------------
                # one padded transpose covers all 4 i-blocks (and one more
                # for the C/S-swapped variant used by the c=2 reads)
                nc.scalar.copy(FEZP[:, :, 0:R], FE[:, 0:NIB, 2, :])
                nc.scalar.copy(FZXP[:, :, 0:Q], FE[:, 0:NIB, 2, Q:R])
                nc.scalar.copy(FZXP[:, :, Q:R], FE[:, 0:NIB, 2, 0:Q])
                tp = ppool2.tile([128, 128], BF16, name="tp", tag="tp")
                nc.tensor.matmul(tp[:], FEZP[:].rearrange("p b q -> p (b q)"),
                                 IDNb[:], is_transpose=True)
                tpx = ppool2.tile([128, 128], BF16, name="tpx", tag="tp")
                nc.tensor.matmul(tpx[:], FZXP[:].rearrange("p b q -> p (b q)"),
                                 IDNb[:], is_transpose=True)
                # per-block copies to base-partition-0 tiles (matmul operands
                # must share a base partition with the rhs)
                zfT = []
                zfTx = []
                for ib in range(NIB):
                    zn = fpool.tile([R, 128], BF16, name=f"zfT{ib}",
                                    tag=f"zfT{ib}")
                    nc.scalar.copy(zn[:], tp[32 * ib:32 * ib + R, :])
                    zfT.append(zn[:])
                    zx = fpool.tile([R, 128], BF16, name=f"zfTx{ib}",
                                    tag=f"zfTx{ib}")
                    nc.scalar.copy(zx[:], tpx[32 * ib:32 * ib + R, :])
                    zfTx.append(zx[:])

                # ---------------- stage 1+2: GXY + S ------------------
                # S[s] [R, RR] accumulates in one PSUM bank (256-wide).
                # DVE produces spin-0 blocks (in chain order) + the spin-1
                # tail; Pool produces the spin-1 head.  Matmuls are emitted
                # in estimated-availability order (the two PSUM accumulate
                # chains are independent, so they interleave freely).
                SPS = [ppool.tile([R, R, R], F32, name=f"S{s}", tag=f"S{s}")
                       for s in range(2)]
                # estimated availability per production schedule
                avail = {}
                for i, b in enumerate(dve_gxy):
                    avail[b] = (i + 1) * 1.0
                for i, b in enumerate(pool_gxy):
                    avail[b] = (i + 1) * 1.85
                seen = {0: 0, 1: 0}
                for b in sorted(avail, key=lambda b: avail[b]):
                    s = 0 if b < NBLK // 2 else 1
                    first = seen[s] == 0
                    last = seen[s] == NBLK // 2 - 1
                    seen[s] += 1
                    nc.tensor.matmul(
                        SPS[s][:], FEZb[:, b, :], gtiles[b][:],
                        start=first, stop=last)

                # ACT copies both spins' S to SBUF so M products can run
                # on either vector engine (Pool cannot read PSUM)
                SSt = [spool.tile([R, 2, Q, R], F32, name=f"SS{s}",
                                  tag=f"SS{s}") for s in range(2)]
                for s in range(2):
                    nc.scalar.copy(SSt[s][:], SPS[s][:].rearrange(
                        "z (hx q) y -> z hx q y", hx=2))
                SS = [SSt[0][:], SSt[1][:]]

                # ---------------- stage 3: M_c ------------------------
                # SS layout [qz, (2,Q)=qx, qy]; W layout [qz, c, qx, qy]
                Mall = spool.tile([R, 3, 2, Q, R], BF16, tag="Mall")
                M = []
                for c in range(3):
                    m = Mall[:, c]
                    t1 = wpool.tile([R, 2, Q, R], F32, name="mt1", tag="mt1")
                    t2 = wpool.tile([R, 2, Q, R], F32, name="mt2", tag="mt2")
                    WAc = WA[:, c].rearrange("z (hx q) y -> z hx q y", hx=2)
                    WBc = WB[:, c].rearrange("z (hx q) y -> z hx q y", hx=2)
                    eng = nc.vector
                    if c == 0:
                        # crossed half read on qx
                        for h in range(2):
                            eng.tensor_tensor(t1[:, h], WAc[:, h], SS[0][:, 1 - h],
                                              AOP.mult)
                            eng.tensor_tensor(t2[:, h], WBc[:, h], SS[1][:, 1 - h],
                                              AOP.mult)
                            eng.tensor_tensor(m[:, h], t1[:, h], t2[:, h], AOP.add)
                    elif c == 1:
                        SSP0 = SSt[0]
                        # crossed read on qy inner (2, Q) split
                        WAc5 = WA[:, c].rearrange(
                            "z x (hy q) -> z x hy q", hy=2)
                        WBc5 = WB[:, c].rearrange(
                            "z x (hy q) -> z x hy q", hy=2)
                        SS05 = [SSP0[:].rearrange("z hx q (hy p) -> z (hx q) hy p", hy=2),
                                SS[1].rearrange("z hx q (hy p) -> z (hx q) hy p", hy=2)]
                        m5 = m.rearrange("z hx q (hy p) -> z (hx q) hy p", hy=2)
                        t15 = t1.rearrange("z hx q (hy p) -> z (hx q) hy p", hy=2)
                        t25 = t2.rearrange("z hx q (hy p) -> z (hx q) hy p", hy=2)
                        for s in range(2):
                            nc.gpsimd.tensor_tensor(t15[:, :, s], WAc5[:, :, s],
                                                    SS05[0][:, :, 1 - s], AOP.mult)
                            nc.vector.tensor_tensor(t25[:, :, s], WBc5[:, :, s],
                                                    SS05[1][:, :, 1 - s], AOP.mult)
                        nc.gpsimd.tensor_tensor(m5[:], t15[:], t25[:], AOP.add)
                    else:
                        eng.tensor_tensor(t1[:], WAc, SS[0][:], AOP.mult)
                        eng.tensor_tensor(t2[:], WBc, SS[1][:], AOP.mult)
                        eng.tensor_tensor(m[:], t1[:], t2[:], AOP.add)
                    M.append(m)

                # ---------------- stage 4: back-transform -------------
                # per i-block: c0+c1 share one T1 matmul (zfT lhsT); c2 uses
                # zfTx; three fused multiply-reduces and one 3-column add
                resall = spool.tile([128, NIB, 3], F32, tag="resall")
                for ib in range(NIB):
                    gi2 = GI[ib][:].rearrange("p a b -> p (a b)")
                    t1a = ppool2.tile([128, 2, RR], F32,
                                      name=f"T1a_{ib}", tag="T1a")
                    nc.tensor.matmul(
                        t1a[:],
                        zfT[ib],
                        Mall[:, 0:2].rearrange("z c h q y -> z (c h q y)"),
                        start=True, stop=True)
                    t1b = ppool2.tile([128, RR], F32,
                                      name=f"T1b_{ib}", tag="T1b")
                    nc.tensor.matmul(
                        t1b[:],
                        zfTx[ib],
                        Mall[:, 2].rearrange("z h q y -> z (h q y)"),
                        start=True, stop=True)
                    accall = spool.tile([128, 3], F32, name=f"acc{ib}",
                                        tag="acc")
                    scr = wpool.tile([128, RR], F32, name="scr", tag="scr")
                    for c in range(3):
                        src_ap = t1a[:, c] if c < 2 else t1b[:]
                        nc.vector.scalar_tensor_tensor(
                            scr[:], src_ap, 0.0, gi2[:],
                            AOP.bypass, AOP.mult,
                            accum_out=accall[:, c:c + 1])
                    nc.vector.tensor_tensor(
                        resall[:, ib], accall[:], RSP[:, ib, :], AOP.add)
                nc.sync.dma_start(out=outp[:, :, :], in_=resall[:])
    return nc


def _split_multi_waits(bir_json: bytes) -> bytes:
    """Walrus rejects >1 sync wait per instruction; hoist extras onto
    same-engine NoOps immediately before (same blocking semantics)."""
    import json as _json
    d = _json.loads(bir_json)
    for fn in d["functions"]:
        for blk in fn["blocks"]:
            new_insts = []
            for inst in blk["instructions"]:
                si = inst.get("sync_info")
                waits = (si or {}).get("on_wait") or []
                if len(waits) > 1:
                    for i, w in enumerate(waits[:-1]):
                        new_insts.append({
                            "debug": inst.get("debug", 0),
                            "engine": inst["engine"],
                            "ins": [], "outs": [],
                            "name": f"{inst['name']}-w{i}",
                            "opcode": "NoOp",
                            "text_hint": "split_wait",
                            "sync_info": {"on_update": [], "on_wait": [w]},
                        })
                    si["on_wait"] = [waits[-1]]
                new_insts.append(inst)
            blk["instructions"] = new_insts
    return _json.dumps(d).encode()


def _get_program(reps=1):
    if reps not in _CACHED:
        nc = _build_program(reps)
        orig = nc.to_json_bytes
        nc.to_json_bytes = lambda: _split_multi_waits(orig())
        _CACHED[reps] = nc
    return _CACHED[reps]


def kernel(rs, same_w1, same_b1, same_wo, same_bo,
           diff_w1, diff_b1, diff_wo, diff_bo):
    global LAST_RESULTS
    in_maps = host_prepare(rs, same_w1, same_b1, same_wo, same_bo,
                           diff_w1, diff_b1, diff_wo, diff_bo)
    for im in in_maps:
        im["repstag"] = np.zeros((1, 1), np.float32)
    nc = _get_program()
    LAST_RESULTS = run_bass_kernel_spmd(nc, in_maps, list(range(NCORES)))
    outs = [np.asarray(LAST_RESULTS.results[i]["out"])   # [128, NIB, 3]
            .transpose(1, 0, 2).reshape(ROWS, 3)
            for i in range(NCORES)]
    return np.concatenate(outs, axis=0).astype(np.float32)
